# revision 24
# baseline (speedup 1.0000x reference)
"""ALiBi causal self-attention on 8 TRN2 NeuronCores, head-sharded (2 heads/core).

Layout/algorithm notes:
  - All matmuls in bf16 with f32 PSUM accumulation.
  - x tiles are cast f32->bf16 in-flight by SWDGE DMA, then transposed on the
    TensorEngine (identity matmuls) to put D on partitions for the
    projections (also keeps the PE HAM-warm from t=0).
  - Scores are computed TRANSPOSED (k on partitions, q free) so that attn@V
    needs no transposes: av = expT.T @ V directly.
  - ALiBi (slope 1.0) + causality make attention banded: probs underflow to
    exact f32 zero beyond ~115 positions from the diagonal, so only a 2-block
    (256-wide) band per 128-row q-block is computed (min coverage Delta<=128,
    and exp(-128+eps) == 0.0f exactly). Output DRAM buffers are donated
    zero-initialized, so the rest of the 512MB attn tensor is free.
  - The full relative-position term (k - q, within batch) rides the QK^T
    matmul as 4 extra contraction rows: (k%128, 1, k-k%128, 1) on the K side
    against (1, -(q%128), 1, -(q-q%128)) on the Q side — each factor is a
    multiple-of-128 or <128 integer, exactly representable in bf16.
  - The 1/sqrt(hd) scale is folded into Wq/bq on the host.
  - Row sums come from a ones-column appended to V (used on-chip to normalize
    av); attn probs are written unnormalized (bf16) and the host divides by
    their own column sums during unshard (exactly consistent).
  - Final projection produces per-core partial sums (bf16); host sums them.
  - Attention is software-pipelined at emission: scores/exp of block i are
    issued before attn@V of block i-1, so the PE never waits on the
    ScalarEngine exp round-trip.
"""

import sys

for _p in ("/opt/trn_rl_repo",):
    if _p not in sys.path:
        sys.path.insert(0, _p)

import numpy as np
import ml_dtypes
from contextlib import ExitStack

import concourse.bass as bass
import concourse.mybir as mybir
import concourse.tile as tile
from concourse import bacc
from concourse.bass import ds, ts
from concourse.bass_utils import run_bass_kernel_spmd

BF16 = mybir.dt.bfloat16
F32 = mybir.dt.float32
AF = mybir.ActivationFunctionType

B, T, D = 2, 2048, 1024
H, HD = 16, 64
NCORES = 8
HPC = H // NCORES          # heads per core = 2
NTOK = B * T               # 4096
NQB = T // 128             # 16 q blocks per batch
NTT = NTOK // 128          # 32 token tiles
BANDB = 2                  # k-blocks in band (incl. diagonal block)
NEG = -1.0e30
NXR = 4                    # extra alibi contraction rows


def build_nc(with_bias=False):
    nc = bacc.Bacc(None, target_bir_lowering=False)

    x = nc.declare_dram_parameter("x", [NTOK, D], F32, isOutput=False)
    wqt = nc.declare_dram_parameter("wqt", [8, 128, 128], BF16, isOutput=False)
    wkt = nc.declare_dram_parameter("wkt", [8, 128, 128], BF16, isOutput=False)
    wvt = nc.declare_dram_parameter("wvt", [8, 128, 128], BF16, isOutput=False)
    wot = nc.declare_dram_parameter("wot", [128, D], BF16, isOutput=False)
    bqr = nc.declare_dram_parameter("bqr", [1, 128], BF16, isOutput=False)
    bkr = nc.declare_dram_parameter("bkr", [1, 128], BF16, isOutput=False)
    bvr = nc.declare_dram_parameter("bvr", [1, 128], BF16, isOutput=False)
    ident = nc.declare_dram_parameter("ident", [128, 128], BF16, isOutput=False)
    causal = nc.declare_dram_parameter("causal", [128, 128], F32, isOutput=False)
    qrows = nc.declare_dram_parameter("qrows", [NXR, NTOK], BF16, isOutput=False)
    krows = nc.declare_dram_parameter("krows", [NXR, NTOK], BF16, isOutput=False)

    out_part = nc.declare_dram_parameter("out_part", [NTOK, D], BF16, isOutput=True)
    attn_t = nc.declare_dram_parameter("attn_t", [B, HPC, T, T], BF16, isOutput=True)

    with tile.TileContext(nc) as tc, ExitStack() as ctx:
        consts = ctx.enter_context(tc.tile_pool(name="consts", bufs=1))
        wpool = ctx.enter_context(tc.tile_pool(name="wpool", bufs=1))
        wstage = ctx.enter_context(tc.tile_pool(name="wstage", bufs=2))
        xstage = ctx.enter_context(tc.tile_pool(name="xstage", bufs=6))
        bigs = ctx.enter_context(tc.tile_pool(name="bigs", bufs=1))
        expp = ctx.enter_context(tc.tile_pool(name="expp", bufs=6))
        avp = ctx.enter_context(tc.tile_pool(name="avp", bufs=3))
        outst = ctx.enter_context(tc.tile_pool(name="outst", bufs=3))
        ps_proj = ctx.enter_context(tc.tile_pool(name="ps_proj", bufs=2, space="PSUM"))
        ps_xt = ctx.enter_context(tc.tile_pool(name="ps_xt", bufs=2, space="PSUM"))
        ps_sc = ctx.enter_context(tc.tile_pool(name="ps_sc", bufs=2, space="PSUM"))
        ps_av = ctx.enter_context(tc.tile_pool(name="ps_av", bufs=1, space="PSUM"))
        ps_avt = ctx.enter_context(tc.tile_pool(name="ps_avt", bufs=1, space="PSUM"))

        # ---- constants ----
        ident_sb = consts.tile([128, 128], BF16, tag="ident")
        nc.sync.dma_start(out=ident_sb[:], in_=ident[:])
        causal_sb = consts.tile([128, 128], F32, tag="causal")
        nc.sync.dma_start(out=causal_sb[:], in_=causal[:])
        bq_sb = consts.tile([1, 128], BF16, tag="bq")
        nc.sync.dma_start(out=bq_sb[:], in_=bqr[:])
        bk_sb = consts.tile([1, 128], BF16, tag="bk")
        nc.sync.dma_start(out=bk_sb[:], in_=bkr[:])
        bv_sb = consts.tile([1, 128], BF16, tag="bv")
        nc.sync.dma_start(out=bv_sb[:], in_=bvr[:])
        ones_row = consts.tile([1, NTOK], BF16, tag="ones")
        nc.vector.memset(ones_row[:], 1.0)

        # ---- big persistent tensors ----
        xt_sb = bigs.tile([128, 8, NTOK], BF16, tag="xt")          # x^T, d-chunk major
        qt_sb = [bigs.tile([64 + NXR, NTOK], BF16, tag=f"qt{h}", name=f"qt{h}")
                 for h in range(HPC)]
        kt_sb = [bigs.tile([64 + NXR, NTOK], BF16, tag=f"kt{h}", name=f"kt{h}")
                 for h in range(HPC)]
        vt_sb = bigs.tile([128, NTOK], BF16, tag="vt")             # V^T (vd on part)
        v_sb = bigs.tile([128, HPC, NTT, 65], BF16, tag="v")       # V + ones col
        avt_sb = bigs.tile([128, NTT, 128], BF16, tag="avt")       # av^T per token tile

        for h in range(HPC):
            nc.sync.dma_start(out=qt_sb[h][64:64 + NXR, :], in_=qrows[:])
            nc.sync.dma_start(out=kt_sb[h][64:64 + NXR, :], in_=krows[:])
        nc.vector.memset(v_sb[:, :, :, 64:65], 1.0)

        # ---- HAM warmup: dummy matmuls fill PE while first x casts run ----
        dummy_sb = consts.tile([128, 512], BF16, tag="dummy")
        nc.vector.memset(dummy_sb[:], 0.0)
        wps = ps_sc.tile([128, 512], F32, tag="sc", name="warm")
        for _ in range(24):
            nc.tensor.matmul(wps[:], ident_sb[:], dummy_sb[:],
                             start=True, stop=True)

        # ---- weights: direct pre-transposed loads ----
        wqT = wpool.tile([128, 8, 128], BF16, tag="wqT")
        nc.sync.dma_start(out=wqT[:], in_=wqt[:].rearrange("k p n -> p k n"))
        wkT = wpool.tile([128, 8, 128], BF16, tag="wkT")
        nc.sync.dma_start(out=wkT[:], in_=wkt[:].rearrange("k p n -> p k n"))
        wvT = wpool.tile([128, 8, 128], BF16, tag="wvT")
        nc.sync.dma_start(out=wvT[:], in_=wvt[:].rearrange("k p n -> p k n"))
        woT = wpool.tile([128, D], BF16, tag="woT")
        nc.sync.dma_start(out=woT[:], in_=wot[:])

        # ---- per-stage emitters ----
        def emit_xdma(tt):
            xs = xstage.tile([128, D], BF16, tag="xs", name=f"xs{tt}")
            nc.gpsimd.dma_start(out=xs[:], in_=x[ts(tt, 128), :])
            return xs

        def emit_xtrans(xs, tt, g):
            pt = ps_xt.tile([128, 512], F32, tag="xtp")
            for j in range(4):
                k = g * 4 + j
                nc.tensor.matmul(pt[:, ts(j, 128)], xs[:, ts(k, 128)],
                                 ident_sb[:], start=True, stop=True)
            eng = nc.scalar.copy if g == 0 else nc.vector.tensor_copy
            eng(xt_sb[:, g * 4:(g + 1) * 4, ts(tt, 128)],
                pt[:].rearrange("p (j n) -> p j n", n=128))

        def emit_qkv(n):  # n-tile: 512 tokens -> QT, KT, VT
            for wT, dst, brow_sb in ((wqT, qt_sb, bq_sb), (wkT, kt_sb, bk_sb),
                                     (wvT, None, bv_sb)):
                pt = ps_proj.tile([128, 512], F32, tag="proj")
                for k in range(8):
                    nc.tensor.matmul(pt[:], wT[:, k, :], xt_sb[:, k, ts(n, 512)],
                                     start=(k == 0), stop=(k == 7 and not with_bias))
                if with_bias:
                    nc.tensor.matmul(pt[:], brow_sb[:], ones_row[:, ts(n, 512)],
                                     start=False, stop=True)
                if dst is None:
                    nc.vector.tensor_copy(vt_sb[:, ts(n, 512)], pt[:])
                else:
                    nc.scalar.copy(dst[0][0:64, ts(n, 512)], pt[0:64, :])
                    nc.vector.tensor_copy(dst[1][0:64, ts(n, 512)], pt[64:128, :])

        def emit_vtile(tt):  # transpose V^T tile -> V [tok, vd] + ones col
            pst = ps_avt.tile([128, 128], F32, tag="avt")
            nc.tensor.matmul(pst[:], vt_sb[:, ts(tt, 128)], ident_sb[:],
                             start=True, stop=True)
            nc.vector.tensor_copy(v_sb[:, 0, tt, 0:64], pst[:, 0:64])
            nc.vector.tensor_copy(v_sb[:, 1, tt, 0:64], pst[:, 64:128])

        def emit_scores(tt):
            """Scores + exp + attn-out DMA for both heads; returns expt tile."""
            b, qb = tt // NQB, tt % NQB
            kb0 = max(0, qb - (BANDB - 1))
            kbs = list(range(kb0, qb + 1))
            nkb = len(kbs)
            sps = ps_sc.tile([128, HPC, BANDB * 128], F32, tag="sc")
            for h in range(HPC):
                for i, kb in enumerate(kbs):
                    nc.tensor.matmul(
                        sps[:, h, ts(i, 128)],
                        kt_sb[h][:, ds(b * T + kb * 128, 128)],
                        qt_sb[h][:, ds(b * T + qb * 128, 128)],
                        start=True, stop=True)
                nc.vector.tensor_add(sps[:, h, ts(nkb - 1, 128)],
                                     sps[:, h, ts(nkb - 1, 128)], causal_sb[:])
            expt = expp.tile([128, HPC, BANDB * 128], BF16, tag="expt")
            nc.scalar.activation(expt[:, :, 0:nkb * 128], sps[:, :, 0:nkb * 128],
                                 AF.Exp, bias=0.0, scale=1.0)
            for h in range(HPC):
                nc.sync.dma_start(
                    out=attn_t[b, h].rearrange("(kb p) q -> p kb q", p=128)
                        [:, kb0:kb0 + nkb, ds(qb * 128, 128)],
                    in_=expt[:, h, 0:nkb * 128].rearrange("p (kb q) -> p kb q",
                                                          q=128))
            return (tt, kbs, expt)

        def emit_avpart(state):
            """attn@V + normalization + av transpose + output projection."""
            tt, kbs, expt = state
            b, qb = tt // NQB, tt % NQB
            av_sb = avp.tile([128, 128], BF16, tag="av_sb")
            avps = ps_av.tile([128, 2, 65], F32, tag="av")
            for h in range(HPC):
                for i, kb in enumerate(kbs):
                    nc.tensor.matmul(avps[:, h, :], expt[:, h, ts(i, 128)],
                                     v_sb[:, h, b * NQB + kb, :],
                                     start=(i == 0), stop=(i == len(kbs) - 1))
                inv = avp.tile([128, 1], F32, tag="inv")
                nc.vector.reciprocal(inv[:], avps[:, h, 64:65])
                nc.vector.tensor_scalar_mul(av_sb[:, ds(h * 64, 64)],
                                            avps[:, h, 0:64], inv[:])
            avtps = ps_avt.tile([128, 128], F32, tag="avt")
            nc.tensor.matmul(avtps[:], av_sb[:], ident_sb[:], start=True, stop=True)
            nc.vector.tensor_copy(avt_sb[:, tt, :], avtps[:])
            # output projection for this token tile
            ot = outst.tile([128, D], BF16, tag="ot")
            for g in range(2):
                pt = ps_proj.tile([128, 512], F32, tag="proj")
                nc.tensor.matmul(pt[:], avt_sb[:, tt, :], woT[:, ts(g, 512)],
                                 start=True, stop=True)
                eng = nc.scalar.copy if g == 0 else nc.vector.tensor_copy
                eng(ot[:, ts(g, 512)], pt[:])
            nc.gpsimd.dma_start(out=out_part[ts(tt, 128), :], in_=ot[:])

        # ---- main pipeline ----
        RUNWAY = 8
        xs_tiles = {}
        for tt in range(RUNWAY):
            xs_tiles[tt] = emit_xdma(tt)
        for tt in range(RUNWAY):
            emit_xtrans(xs_tiles[tt], tt, 0)
            emit_xtrans(xs_tiles[tt], tt, 1)
            del xs_tiles[tt]
        prev = None
        for n in range(8):
            emit_qkv(n)
            for tt in range(4 * n, 4 * n + 4):
                ft = tt + RUNWAY
                if ft < NTT:
                    xs_tiles[ft] = emit_xdma(ft)
                emit_vtile(tt)
                if ft < NTT:
                    emit_xtrans(xs_tiles[ft], ft, 0)
                state = emit_scores(tt)
                if ft < NTT:
                    emit_xtrans(xs_tiles[ft], ft, 1)
                    del xs_tiles[ft]
                if prev is not None:
                    emit_avpart(prev)
                prev = state
        emit_avpart(prev)

    nc.compile()
    return nc


_NC_CACHE = {}


def _get_nc(with_bias):
    if with_bias not in _NC_CACHE:
        _NC_CACHE[with_bias] = build_nc(with_bias)
    return _NC_CACHE[with_bias]


def _make_in_maps(x, Wq, bq, Wk, bk, Wv, bv, Wo, bo):
    bf = ml_dtypes.bfloat16
    x2d = np.ascontiguousarray(np.asarray(x, np.float32).reshape(NTOK, D))
    t = np.arange(NTOK)
    tl = (t % 128).astype(np.float32)              # local position
    tb = ((t % T) - (t % 128)).astype(np.float32)  # 128*block within batch
    one = np.ones(NTOK, np.float32)
    qrows = np.stack([one, -tl, one, -tb]).astype(bf)
    krows = np.stack([tl, one, tb, one]).astype(bf)
    ident = np.eye(128, dtype=np.float32).astype(bf)
    kq = np.arange(128)
    causal = np.where(kq[:, None] <= kq[None, :], 0.0, NEG).astype(np.float32)
    scale = 1.0 / np.sqrt(np.float32(HD))

    in_maps = []
    for c in range(NCORES):
        r = slice(c * 128, (c + 1) * 128)
        in_maps.append({
            "x": x2d,
            "wqt": np.ascontiguousarray(
                (np.asarray(Wq, np.float32)[r, :] * scale).T).astype(bf)
                .reshape(8, 128, 128),
            "wkt": np.ascontiguousarray(
                np.asarray(Wk, np.float32)[r, :].T).astype(bf).reshape(8, 128, 128),
            "wvt": np.ascontiguousarray(
                np.asarray(Wv, np.float32)[r, :].T).astype(bf).reshape(8, 128, 128),
            "wot": np.ascontiguousarray(
                np.asarray(Wo, np.float32)[:, r].T).astype(bf),
            "bqr": (np.asarray(bq, np.float32)[r] * scale).reshape(1, 128).astype(bf),
            "bkr": np.asarray(bk, np.float32)[r].reshape(1, 128).astype(bf),
            "bvr": np.asarray(bv, np.float32)[r].reshape(1, 128).astype(bf),
            "ident": ident,
            "causal": causal,
            "qrows": qrows,
            "krows": krows,
        })
    return in_maps


def _assemble(results, bo):
    out = np.zeros((NTOK, D), np.float32)
    for c in range(NCORES):
        out += results[c]["out_part"].astype(np.float32)
    out += np.asarray(bo, np.float32)[None, :]
    out = out.reshape(B, T, D)

    attn = np.zeros((B, H, T, T), np.float32)
    for c in range(NCORES):
        et = results[c]["attn_t"]      # [B, HPC, T(k), T(q)] bf16, unnormalized
        for b in range(B):
            for hh in range(HPC):
                probs_t = et[b, hh].astype(np.float32)
                probs_t /= probs_t.sum(axis=0)[None, :]
                attn[b, c * HPC + hh] = probs_t.T
    return out, attn


def kernel(x, Wq, bq, Wk, bk, Wv, bv, Wo, bo, _trace=False):
    with_bias = bool(np.any(np.asarray(bq)) or np.any(np.asarray(bk))
                     or np.any(np.asarray(bv)))
    nc = _get_nc(with_bias)
    in_maps = _make_in_maps(x, Wq, bq, Wk, bk, Wv, bv, Wo, bo)
    res = run_bass_kernel_spmd(nc, in_maps, core_ids=list(range(NCORES)),
                               trace=_trace)
    out, attn = _assemble(res.results, bo)
    if _trace:
        kernel.last_exec_time_ns = res.exec_time_ns
        kernel.last_result = res
    return out, attn


# revision 25
# speedup vs baseline: 1.2336x; 1.2336x over previous
"""ALiBi causal self-attention on 8 TRN2 NeuronCores, head-sharded (2 heads/core).

Layout/algorithm notes:
  - All matmuls in bf16 with f32 PSUM accumulation.
  - x tiles are cast f32->bf16 in-flight by SWDGE DMA, then transposed on the
    TensorEngine (identity matmuls) to put D on partitions for the
    projections (also keeps the PE HAM-warm from t=0).
  - Scores are computed TRANSPOSED (k on partitions, q free) so that attn@V
    needs no transposes: av = expT.T @ V directly.
  - ALiBi (slope 1.0) + causality make attention banded: probs underflow to
    exact f32 zero beyond ~115 positions from the diagonal, so only a 2-block
    (256-wide) band per 128-row q-block is computed (min coverage Delta<=128,
    and exp(-128+eps) == 0.0f exactly). Output DRAM buffers are donated
    zero-initialized, so the rest of the 512MB attn tensor is free.
  - The full relative-position term (k - q, within batch) rides the QK^T
    matmul as 4 extra contraction rows: (k%128, 1, k-k%128, 1) on the K side
    against (1, -(q%128), 1, -(q-q%128)) on the Q side — each factor is a
    multiple-of-128 or <128 integer, exactly representable in bf16.
  - The 1/sqrt(hd) scale is folded into Wq/bq on the host.
  - Row sums come from a ones-column appended to V (used on-chip to normalize
    av); attn probs are written unnormalized (bf16) and the host divides by
    their own column sums during unshard (exactly consistent).
  - Final projection produces per-core partial sums (bf16); host sums them.
  - Attention is software-pipelined at emission: scores/exp of block i are
    issued before attn@V of block i-1, so the PE never waits on the
    ScalarEngine exp round-trip.
"""

import sys

for _p in ("/opt/trn_rl_repo",):
    if _p not in sys.path:
        sys.path.insert(0, _p)

import numpy as np
import ml_dtypes
from contextlib import ExitStack

import concourse.bass as bass
import concourse.mybir as mybir
import concourse.tile as tile
from concourse import bacc
from concourse.bass import ds, ts
from concourse.bass_utils import run_bass_kernel_spmd

BF16 = mybir.dt.bfloat16
F32 = mybir.dt.float32
AF = mybir.ActivationFunctionType

B, T, D = 2, 2048, 1024
H, HD = 16, 64
NCORES = 8
HPC = H // NCORES          # heads per core = 2
NTOK = B * T               # 4096
NQB = T // 128             # 16 q blocks per batch
NTT = NTOK // 128          # 32 token tiles
BANDB = 2                  # k-blocks in band (incl. diagonal block)
NEG = -1.0e30
NXR = 4                    # extra alibi contraction rows


def build_nc(with_bias=False):
    nc = bacc.Bacc(None, target_bir_lowering=False)

    x = nc.declare_dram_parameter("x", [NTOK, D], F32, isOutput=False)
    wqt = nc.declare_dram_parameter("wqt", [8, 128, 128], BF16, isOutput=False)
    wkt = nc.declare_dram_parameter("wkt", [8, 128, 128], BF16, isOutput=False)
    wvt = nc.declare_dram_parameter("wvt", [8, 128, 128], BF16, isOutput=False)
    wot = nc.declare_dram_parameter("wot", [128, D], BF16, isOutput=False)
    bqr = nc.declare_dram_parameter("bqr", [1, 128], BF16, isOutput=False)
    bkr = nc.declare_dram_parameter("bkr", [1, 128], BF16, isOutput=False)
    bvr = nc.declare_dram_parameter("bvr", [1, 128], BF16, isOutput=False)
    ident = nc.declare_dram_parameter("ident", [128, 128], BF16, isOutput=False)
    causal = nc.declare_dram_parameter("causal", [128, 128], F32, isOutput=False)
    qrows = nc.declare_dram_parameter("qrows", [NXR, NTOK], BF16, isOutput=False)
    krows = nc.declare_dram_parameter("krows", [NXR, NTOK], BF16, isOutput=False)

    out_part = nc.declare_dram_parameter("out_part", [NTOK, D], BF16, isOutput=True)
    attn_t = nc.declare_dram_parameter("attn_t", [B, HPC, T, T], BF16, isOutput=True)

    with tile.TileContext(nc) as tc, ExitStack() as ctx:
        consts = ctx.enter_context(tc.tile_pool(name="consts", bufs=1))
        wpool = ctx.enter_context(tc.tile_pool(name="wpool", bufs=1))
        wstage = ctx.enter_context(tc.tile_pool(name="wstage", bufs=2))
        xstage = ctx.enter_context(tc.tile_pool(name="xstage", bufs=6))
        bigs = ctx.enter_context(tc.tile_pool(name="bigs", bufs=1))
        expp = ctx.enter_context(tc.tile_pool(name="expp", bufs=6))
        avp = ctx.enter_context(tc.tile_pool(name="avp", bufs=3))
        outst = ctx.enter_context(tc.tile_pool(name="outst", bufs=3))
        ps_proj = ctx.enter_context(tc.tile_pool(name="ps_proj", bufs=2, space="PSUM"))
        ps_xt = ctx.enter_context(tc.tile_pool(name="ps_xt", bufs=2, space="PSUM"))
        ps_sc = ctx.enter_context(tc.tile_pool(name="ps_sc", bufs=2, space="PSUM"))
        ps_av = ctx.enter_context(tc.tile_pool(name="ps_av", bufs=1, space="PSUM"))
        ps_avt = ctx.enter_context(tc.tile_pool(name="ps_avt", bufs=1, space="PSUM"))

        # ---- constants ----
        ident_sb = consts.tile([128, 128], BF16, tag="ident")
        nc.sync.dma_start(out=ident_sb[:], in_=ident[:])
        causal_sb = consts.tile([128, 128], F32, tag="causal")
        nc.sync.dma_start(out=causal_sb[:], in_=causal[:])
        bq_sb = consts.tile([1, 128], BF16, tag="bq")
        nc.sync.dma_start(out=bq_sb[:], in_=bqr[:])
        bk_sb = consts.tile([1, 128], BF16, tag="bk")
        nc.sync.dma_start(out=bk_sb[:], in_=bkr[:])
        bv_sb = consts.tile([1, 128], BF16, tag="bv")
        nc.sync.dma_start(out=bv_sb[:], in_=bvr[:])
        ones_row = consts.tile([1, NTOK], BF16, tag="ones")
        nc.vector.memset(ones_row[:], 1.0)

        # ---- big persistent tensors ----
        xt_sb = bigs.tile([128, 8, NTOK], BF16, tag="xt")          # x^T, d-chunk major
        qt_sb = [bigs.tile([64 + NXR, NTOK], BF16, tag=f"qt{h}", name=f"qt{h}")
                 for h in range(HPC)]
        kt_sb = [bigs.tile([64 + NXR, NTOK], BF16, tag=f"kt{h}", name=f"kt{h}")
                 for h in range(HPC)]
        vt_sb = bigs.tile([128, NTOK], BF16, tag="vt")             # V^T (vd on part)
        v_sb = bigs.tile([128, HPC, NTT, 65], BF16, tag="v")       # V + ones col
        avt_sb = bigs.tile([128, NTT, 128], BF16, tag="avt")       # av^T per token tile

        for h in range(HPC):
            nc.sync.dma_start(out=qt_sb[h][64:64 + NXR, :], in_=qrows[:])
            nc.sync.dma_start(out=kt_sb[h][64:64 + NXR, :], in_=krows[:])
        nc.vector.memset(v_sb[:, :, :, 64:65], 1.0)

        # ---- HAM warmup: dummy matmuls fill PE while first x casts run ----
        dummy_sb = consts.tile([128, 512], BF16, tag="dummy")
        nc.vector.memset(dummy_sb[:], 0.0)
        wps = ps_sc.tile([128, 512], F32, tag="sc", name="warm")
        for _ in range(24):
            nc.tensor.matmul(wps[:], ident_sb[:], dummy_sb[:],
                             start=True, stop=True)

        # ---- weights: direct pre-transposed loads ----
        wqT = wpool.tile([128, 8, 128], BF16, tag="wqT")
        nc.sync.dma_start(out=wqT[:], in_=wqt[:].rearrange("k p n -> p k n"))
        wkT = wpool.tile([128, 8, 128], BF16, tag="wkT")
        nc.sync.dma_start(out=wkT[:], in_=wkt[:].rearrange("k p n -> p k n"))
        wvT = wpool.tile([128, 8, 128], BF16, tag="wvT")
        nc.sync.dma_start(out=wvT[:], in_=wvt[:].rearrange("k p n -> p k n"))
        woT = wpool.tile([128, D], BF16, tag="woT")
        nc.sync.dma_start(out=woT[:], in_=wot[:])

        # ---- per-stage emitters ----
        def emit_xdma(tt):
            xs = xstage.tile([128, D], BF16, tag="xs", name=f"xs{tt}")
            nc.gpsimd.dma_start(out=xs[:], in_=x[ts(tt, 128), :])
            return xs

        def emit_xtrans(xs, tt, g):
            pt = ps_xt.tile([128, 512], F32, tag="xtp")
            for j in range(4):
                k = g * 4 + j
                nc.tensor.matmul(pt[:, ts(j, 128)], xs[:, ts(k, 128)],
                                 ident_sb[:], start=True, stop=True)
            eng = nc.scalar.copy if g == 0 else nc.vector.tensor_copy
            eng(xt_sb[:, g * 4:(g + 1) * 4, ts(tt, 128)],
                pt[:].rearrange("p (j n) -> p j n", n=128))

        def emit_qkv(n):  # n-tile: 512 tokens -> QT, KT, VT
            for wT, dst, brow_sb in ((wqT, qt_sb, bq_sb), (wkT, kt_sb, bk_sb),
                                     (wvT, None, bv_sb)):
                pt = ps_proj.tile([128, 512], F32, tag="proj")
                for k in range(8):
                    nc.tensor.matmul(pt[:], wT[:, k, :], xt_sb[:, k, ts(n, 512)],
                                     start=(k == 0), stop=(k == 7 and not with_bias))
                if with_bias:
                    nc.tensor.matmul(pt[:], brow_sb[:], ones_row[:, ts(n, 512)],
                                     start=False, stop=True)
                if dst is None:
                    nc.vector.tensor_copy(vt_sb[:, ts(n, 512)], pt[:])
                else:
                    nc.scalar.copy(dst[0][0:64, ts(n, 512)], pt[0:64, :])
                    nc.vector.tensor_copy(dst[1][0:64, ts(n, 512)], pt[64:128, :])

        def emit_vtile(tt):  # transpose V^T tile -> V [tok, vd] + ones col
            pst = ps_avt.tile([128, 128], F32, tag="avt")
            nc.tensor.matmul(pst[:], vt_sb[:, ts(tt, 128)], ident_sb[:],
                             start=True, stop=True)
            nc.vector.tensor_copy(v_sb[:, 0, tt, 0:64], pst[:, 0:64])
            nc.vector.tensor_copy(v_sb[:, 1, tt, 0:64], pst[:, 64:128])

        def emit_scores(tt):
            """Scores + exp + attn-out DMA for both heads; returns expt tile."""
            b, qb = tt // NQB, tt % NQB
            kb0 = max(0, qb - (BANDB - 1))
            kbs = list(range(kb0, qb + 1))
            nkb = len(kbs)
            sps = ps_sc.tile([128, HPC, BANDB * 128], F32, tag="sc")
            for h in range(HPC):
                for i, kb in enumerate(kbs):
                    nc.tensor.matmul(
                        sps[:, h, ts(i, 128)],
                        kt_sb[h][:, ds(b * T + kb * 128, 128)],
                        qt_sb[h][:, ds(b * T + qb * 128, 128)],
                        start=True, stop=True)
                nc.vector.tensor_add(sps[:, h, ts(nkb - 1, 128)],
                                     sps[:, h, ts(nkb - 1, 128)], causal_sb[:])
            expt = expp.tile([128, HPC, BANDB * 128], BF16, tag="expt")
            nc.scalar.activation(expt[:, :, 0:nkb * 128], sps[:, :, 0:nkb * 128],
                                 AF.Exp, bias=0.0, scale=1.0)
            for h in range(HPC):
                nc.sync.dma_start(
                    out=attn_t[b, h].rearrange("(kb p) q -> p kb q", p=128)
                        [:, kb0:kb0 + nkb, ds(qb * 128, 128)],
                    in_=expt[:, h, 0:nkb * 128].rearrange("p (kb q) -> p kb q",
                                                          q=128))
            return (tt, kbs, expt)

        def emit_avpart(state):
            """attn@V + normalization + av transpose + output projection."""
            tt, kbs, expt = state
            b, qb = tt // NQB, tt % NQB
            av_sb = avp.tile([128, 128], BF16, tag="av_sb")
            avps = ps_av.tile([128, 2, 65], F32, tag="av")
            for h in range(HPC):
                for i, kb in enumerate(kbs):
                    nc.tensor.matmul(avps[:, h, :], expt[:, h, ts(i, 128)],
                                     v_sb[:, h, b * NQB + kb, :],
                                     start=(i == 0), stop=(i == len(kbs) - 1))
                inv = avp.tile([128, 1], F32, tag="inv")
                nc.vector.reciprocal(inv[:], avps[:, h, 64:65])
                nc.vector.tensor_scalar_mul(av_sb[:, ds(h * 64, 64)],
                                            avps[:, h, 0:64], inv[:])
            avtps = ps_avt.tile([128, 128], F32, tag="avt")
            nc.tensor.matmul(avtps[:], av_sb[:], ident_sb[:], start=True, stop=True)
            nc.vector.tensor_copy(avt_sb[:, tt, :], avtps[:])
            # output projection for this token tile
            ot = outst.tile([128, D], BF16, tag="ot")
            for g in range(2):
                pt = ps_proj.tile([128, 512], F32, tag="proj")
                nc.tensor.matmul(pt[:], avt_sb[:, tt, :], woT[:, ts(g, 512)],
                                 start=True, stop=True)
                eng = nc.scalar.copy if g == 0 else nc.vector.tensor_copy
                eng(ot[:, ts(g, 512)], pt[:])
            nc.scalar.dma_start(out=out_part[ts(tt, 128), :], in_=ot[:])

        # ---- main pipeline ----
        RUNWAY = 8
        xs_tiles = {}
        for tt in range(RUNWAY):
            xs_tiles[tt] = emit_xdma(tt)
        for tt in range(RUNWAY):
            emit_xtrans(xs_tiles[tt], tt, 0)
            emit_xtrans(xs_tiles[tt], tt, 1)
            del xs_tiles[tt]
        prev = None
        for n in range(8):
            emit_qkv(n)
            for tt in range(4 * n, 4 * n + 4):
                ft = tt + RUNWAY
                if ft < NTT:
                    xs_tiles[ft] = emit_xdma(ft)
                emit_vtile(tt)
                if ft < NTT:
                    emit_xtrans(xs_tiles[ft], ft, 0)
                state = emit_scores(tt)
                if ft < NTT:
                    emit_xtrans(xs_tiles[ft], ft, 1)
                    del xs_tiles[ft]
                if prev is not None:
                    emit_avpart(prev)
                prev = state
        emit_avpart(prev)

    nc.compile()
    return nc


_NC_CACHE = {}


def _get_nc(with_bias):
    if with_bias not in _NC_CACHE:
        _NC_CACHE[with_bias] = build_nc(with_bias)
    return _NC_CACHE[with_bias]


def _make_in_maps(x, Wq, bq, Wk, bk, Wv, bv, Wo, bo):
    bf = ml_dtypes.bfloat16
    x2d = np.ascontiguousarray(np.asarray(x, np.float32).reshape(NTOK, D))
    t = np.arange(NTOK)
    tl = (t % 128).astype(np.float32)              # local position
    tb = ((t % T) - (t % 128)).astype(np.float32)  # 128*block within batch
    one = np.ones(NTOK, np.float32)
    qrows = np.stack([one, -tl, one, -tb]).astype(bf)
    krows = np.stack([tl, one, tb, one]).astype(bf)
    ident = np.eye(128, dtype=np.float32).astype(bf)
    kq = np.arange(128)
    causal = np.where(kq[:, None] <= kq[None, :], 0.0, NEG).astype(np.float32)
    scale = 1.0 / np.sqrt(np.float32(HD))

    in_maps = []
    for c in range(NCORES):
        r = slice(c * 128, (c + 1) * 128)
        in_maps.append({
            "x": x2d,
            "wqt": np.ascontiguousarray(
                (np.asarray(Wq, np.float32)[r, :] * scale).T).astype(bf)
                .reshape(8, 128, 128),
            "wkt": np.ascontiguousarray(
                np.asarray(Wk, np.float32)[r, :].T).astype(bf).reshape(8, 128, 128),
            "wvt": np.ascontiguousarray(
                np.asarray(Wv, np.float32)[r, :].T).astype(bf).reshape(8, 128, 128),
            "wot": np.ascontiguousarray(
                np.asarray(Wo, np.float32)[:, r].T).astype(bf),
            "bqr": (np.asarray(bq, np.float32)[r] * scale).reshape(1, 128).astype(bf),
            "bkr": np.asarray(bk, np.float32)[r].reshape(1, 128).astype(bf),
            "bvr": np.asarray(bv, np.float32)[r].reshape(1, 128).astype(bf),
            "ident": ident,
            "causal": causal,
            "qrows": qrows,
            "krows": krows,
        })
    return in_maps


def _assemble(results, bo):
    out = np.zeros((NTOK, D), np.float32)
    for c in range(NCORES):
        out += results[c]["out_part"].astype(np.float32)
    out += np.asarray(bo, np.float32)[None, :]
    out = out.reshape(B, T, D)

    attn = np.zeros((B, H, T, T), np.float32)
    for c in range(NCORES):
        et = results[c]["attn_t"]      # [B, HPC, T(k), T(q)] bf16, unnormalized
        for b in range(B):
            for hh in range(HPC):
                probs_t = et[b, hh].astype(np.float32)
                probs_t /= probs_t.sum(axis=0)[None, :]
                attn[b, c * HPC + hh] = probs_t.T
    return out, attn


def kernel(x, Wq, bq, Wk, bk, Wv, bv, Wo, bo, _trace=False):
    with_bias = bool(np.any(np.asarray(bq)) or np.any(np.asarray(bk))
                     or np.any(np.asarray(bv)))
    nc = _get_nc(with_bias)
    in_maps = _make_in_maps(x, Wq, bq, Wk, bk, Wv, bv, Wo, bo)
    res = run_bass_kernel_spmd(nc, in_maps, core_ids=list(range(NCORES)),
                               trace=_trace)
    out, attn = _assemble(res.results, bo)
    if _trace:
        kernel.last_exec_time_ns = res.exec_time_ns
        kernel.last_result = res
    return out, attn


# revision 26
# speedup vs baseline: 1.2429x; 1.0075x over previous
"""ALiBi causal self-attention on 8 TRN2 NeuronCores, head-sharded (2 heads/core).

Layout/algorithm notes:
  - All matmuls in bf16 with f32 PSUM accumulation.
  - x tiles are cast f32->bf16 in-flight by SWDGE DMA, then transposed on the
    TensorEngine (identity matmuls) to put D on partitions for the
    projections (also keeps the PE HAM-warm from t=0).
  - Scores are computed TRANSPOSED (k on partitions, q free) so that attn@V
    needs no transposes: av = expT.T @ V directly.
  - ALiBi (slope 1.0) + causality make attention banded: probs underflow to
    exact f32 zero beyond ~115 positions from the diagonal, so only a 2-block
    (256-wide) band per 128-row q-block is computed (min coverage Delta<=128,
    and exp(-128+eps) == 0.0f exactly). Output DRAM buffers are donated
    zero-initialized, so the rest of the 512MB attn tensor is free.
  - The full relative-position term (k - q, within batch) rides the QK^T
    matmul as 4 extra contraction rows: (k%128, 1, k-k%128, 1) on the K side
    against (1, -(q%128), 1, -(q-q%128)) on the Q side — each factor is a
    multiple-of-128 or <128 integer, exactly representable in bf16.
  - The 1/sqrt(hd) scale is folded into Wq/bq on the host.
  - Row sums come from a ones-column appended to V (used on-chip to normalize
    av); attn probs are written unnormalized (bf16) and the host divides by
    their own column sums during unshard (exactly consistent).
  - Final projection produces per-core partial sums (bf16); host sums them.
  - Attention is software-pipelined at emission: scores/exp of block i are
    issued before attn@V of block i-1, so the PE never waits on the
    ScalarEngine exp round-trip.
"""

import sys

for _p in ("/opt/trn_rl_repo",):
    if _p not in sys.path:
        sys.path.insert(0, _p)

import numpy as np
import ml_dtypes
from contextlib import ExitStack

import concourse.bass as bass
import concourse.mybir as mybir
import concourse.tile as tile
from concourse import bacc
from concourse.bass import ds, ts
from concourse.bass_utils import run_bass_kernel_spmd

BF16 = mybir.dt.bfloat16
F32 = mybir.dt.float32
AF = mybir.ActivationFunctionType

B, T, D = 2, 2048, 1024
H, HD = 16, 64
NCORES = 8
HPC = H // NCORES          # heads per core = 2
NTOK = B * T               # 4096
NQB = T // 128             # 16 q blocks per batch
NTT = NTOK // 128          # 32 token tiles
BANDB = 2                  # k-blocks in band (incl. diagonal block)
NEG = -1.0e30
NXR = 4                    # extra alibi contraction rows


def build_nc(with_bias=False):
    nc = bacc.Bacc(None, target_bir_lowering=False)

    x = nc.declare_dram_parameter("x", [NTOK, D], F32, isOutput=False)
    wqt = nc.declare_dram_parameter("wqt", [8, 128, 128], BF16, isOutput=False)
    wkt = nc.declare_dram_parameter("wkt", [8, 128, 128], BF16, isOutput=False)
    wvt = nc.declare_dram_parameter("wvt", [8, 128, 128], BF16, isOutput=False)
    wot = nc.declare_dram_parameter("wot", [128, D], BF16, isOutput=False)
    bqr = nc.declare_dram_parameter("bqr", [1, 128], BF16, isOutput=False)
    bkr = nc.declare_dram_parameter("bkr", [1, 128], BF16, isOutput=False)
    bvr = nc.declare_dram_parameter("bvr", [1, 128], BF16, isOutput=False)
    ident = nc.declare_dram_parameter("ident", [128, 128], BF16, isOutput=False)
    causal = nc.declare_dram_parameter("causal", [128, 128], F32, isOutput=False)
    qrows = nc.declare_dram_parameter("qrows", [NXR, NTOK], BF16, isOutput=False)
    krows = nc.declare_dram_parameter("krows", [NXR, NTOK], BF16, isOutput=False)

    out_part = nc.declare_dram_parameter("out_part", [NTOK, D], BF16, isOutput=True)
    attn_t = nc.declare_dram_parameter("attn_t", [B, HPC, T, T], BF16, isOutput=True)

    with tile.TileContext(nc) as tc, ExitStack() as ctx:
        consts = ctx.enter_context(tc.tile_pool(name="consts", bufs=1))
        wpool = ctx.enter_context(tc.tile_pool(name="wpool", bufs=1))
        wstage = ctx.enter_context(tc.tile_pool(name="wstage", bufs=2))
        xstage = ctx.enter_context(tc.tile_pool(name="xstage", bufs=6))
        bigs = ctx.enter_context(tc.tile_pool(name="bigs", bufs=1))
        expp = ctx.enter_context(tc.tile_pool(name="expp", bufs=6))
        avp = ctx.enter_context(tc.tile_pool(name="avp", bufs=3))
        outst = ctx.enter_context(tc.tile_pool(name="outst", bufs=3))
        ps_proj = ctx.enter_context(tc.tile_pool(name="ps_proj", bufs=2, space="PSUM"))
        ps_xt = ctx.enter_context(tc.tile_pool(name="ps_xt", bufs=2, space="PSUM"))
        ps_sc = ctx.enter_context(tc.tile_pool(name="ps_sc", bufs=2, space="PSUM"))
        ps_av = ctx.enter_context(tc.tile_pool(name="ps_av", bufs=1, space="PSUM"))
        ps_avt = ctx.enter_context(tc.tile_pool(name="ps_avt", bufs=1, space="PSUM"))

        # ---- constants ----
        ident_sb = consts.tile([128, 128], BF16, tag="ident")
        nc.sync.dma_start(out=ident_sb[:], in_=ident[:])
        causal_sb = consts.tile([128, 128], F32, tag="causal")
        nc.sync.dma_start(out=causal_sb[:], in_=causal[:])
        bq_sb = consts.tile([1, 128], BF16, tag="bq")
        nc.sync.dma_start(out=bq_sb[:], in_=bqr[:])
        bk_sb = consts.tile([1, 128], BF16, tag="bk")
        nc.sync.dma_start(out=bk_sb[:], in_=bkr[:])
        bv_sb = consts.tile([1, 128], BF16, tag="bv")
        nc.sync.dma_start(out=bv_sb[:], in_=bvr[:])
        ones_row = consts.tile([1, NTOK], BF16, tag="ones")
        nc.vector.memset(ones_row[:], 1.0)

        # ---- big persistent tensors ----
        xt_sb = bigs.tile([128, 8, NTOK], BF16, tag="xt")          # x^T, d-chunk major
        qt_sb = [bigs.tile([64 + NXR, NTOK], BF16, tag=f"qt{h}", name=f"qt{h}")
                 for h in range(HPC)]
        kt_sb = [bigs.tile([64 + NXR, NTOK], BF16, tag=f"kt{h}", name=f"kt{h}")
                 for h in range(HPC)]
        vt_sb = bigs.tile([128, NTOK], BF16, tag="vt")             # V^T (vd on part)
        v_sb = bigs.tile([128, HPC, NTT, 65], BF16, tag="v")       # V + ones col
        avt_sb = bigs.tile([128, NTT, 128], BF16, tag="avt")       # av^T per token tile

        for h in range(HPC):
            nc.sync.dma_start(out=qt_sb[h][64:64 + NXR, :], in_=qrows[:])
            nc.sync.dma_start(out=kt_sb[h][64:64 + NXR, :], in_=krows[:])
        nc.vector.memset(v_sb[:, :, :, 64:65], 1.0)

        # ---- HAM warmup: dummy matmuls fill PE while first x casts run ----
        dummy_sb = consts.tile([128, 512], BF16, tag="dummy")
        nc.vector.memset(dummy_sb[:], 0.0)
        wps = ps_sc.tile([128, 512], F32, tag="sc", name="warm")
        for _ in range(48):
            nc.tensor.matmul(wps[:], ident_sb[:], dummy_sb[:],
                             start=True, stop=True)

        # ---- weights: direct pre-transposed loads ----
        wqT = wpool.tile([128, 8, 128], BF16, tag="wqT")
        nc.sync.dma_start(out=wqT[:], in_=wqt[:].rearrange("k p n -> p k n"))
        wkT = wpool.tile([128, 8, 128], BF16, tag="wkT")
        nc.sync.dma_start(out=wkT[:], in_=wkt[:].rearrange("k p n -> p k n"))
        wvT = wpool.tile([128, 8, 128], BF16, tag="wvT")
        nc.sync.dma_start(out=wvT[:], in_=wvt[:].rearrange("k p n -> p k n"))
        woT = wpool.tile([128, D], BF16, tag="woT")
        nc.sync.dma_start(out=woT[:], in_=wot[:])

        # ---- per-stage emitters ----
        def emit_xdma(tt):
            xs = xstage.tile([128, D], BF16, tag="xs", name=f"xs{tt}")
            nc.gpsimd.dma_start(out=xs[:], in_=x[ts(tt, 128), :])
            return xs

        def emit_xtrans(xs, tt, g):
            pt = ps_xt.tile([128, 512], F32, tag="xtp")
            for j in range(4):
                k = g * 4 + j
                nc.tensor.matmul(pt[:, ts(j, 128)], xs[:, ts(k, 128)],
                                 ident_sb[:], start=True, stop=True)
            eng = nc.scalar.copy if g == 0 else nc.vector.tensor_copy
            eng(xt_sb[:, g * 4:(g + 1) * 4, ts(tt, 128)],
                pt[:].rearrange("p (j n) -> p j n", n=128))

        def emit_qkv(n):  # n-tile: 512 tokens -> QT, KT, VT
            for wT, dst, brow_sb in ((wqT, qt_sb, bq_sb), (wkT, kt_sb, bk_sb),
                                     (wvT, None, bv_sb)):
                pt = ps_proj.tile([128, 512], F32, tag="proj")
                for k in range(8):
                    nc.tensor.matmul(pt[:], wT[:, k, :], xt_sb[:, k, ts(n, 512)],
                                     start=(k == 0), stop=(k == 7 and not with_bias))
                if with_bias:
                    nc.tensor.matmul(pt[:], brow_sb[:], ones_row[:, ts(n, 512)],
                                     start=False, stop=True)
                if dst is None:
                    nc.vector.tensor_copy(vt_sb[:, ts(n, 512)], pt[:])
                else:
                    nc.scalar.copy(dst[0][0:64, ts(n, 512)], pt[0:64, :])
                    nc.vector.tensor_copy(dst[1][0:64, ts(n, 512)], pt[64:128, :])

        def emit_vtile(tt):  # transpose V^T tile -> V [tok, vd] + ones col
            pst = ps_avt.tile([128, 128], F32, tag="avt")
            nc.tensor.matmul(pst[:], vt_sb[:, ts(tt, 128)], ident_sb[:],
                             start=True, stop=True)
            nc.vector.tensor_copy(v_sb[:, 0, tt, 0:64], pst[:, 0:64])
            nc.vector.tensor_copy(v_sb[:, 1, tt, 0:64], pst[:, 64:128])

        def emit_scores(tt):
            """Scores + exp + attn-out DMA for both heads; returns expt tile."""
            b, qb = tt // NQB, tt % NQB
            kb0 = max(0, qb - (BANDB - 1))
            kbs = list(range(kb0, qb + 1))
            nkb = len(kbs)
            sps = ps_sc.tile([128, HPC, BANDB * 128], F32, tag="sc")
            for h in range(HPC):
                for i, kb in enumerate(kbs):
                    nc.tensor.matmul(
                        sps[:, h, ts(i, 128)],
                        kt_sb[h][:, ds(b * T + kb * 128, 128)],
                        qt_sb[h][:, ds(b * T + qb * 128, 128)],
                        start=True, stop=True)
                nc.vector.tensor_add(sps[:, h, ts(nkb - 1, 128)],
                                     sps[:, h, ts(nkb - 1, 128)], causal_sb[:])
            expt = expp.tile([128, HPC, BANDB * 128], BF16, tag="expt")
            nc.scalar.activation(expt[:, :, 0:nkb * 128], sps[:, :, 0:nkb * 128],
                                 AF.Exp, bias=0.0, scale=1.0)
            for h in range(HPC):
                nc.sync.dma_start(
                    out=attn_t[b, h].rearrange("(kb p) q -> p kb q", p=128)
                        [:, kb0:kb0 + nkb, ds(qb * 128, 128)],
                    in_=expt[:, h, 0:nkb * 128].rearrange("p (kb q) -> p kb q",
                                                          q=128))
            return (tt, kbs, expt)

        def emit_avpart(state):
            """attn@V + normalization + av transpose + output projection."""
            tt, kbs, expt = state
            b, qb = tt // NQB, tt % NQB
            av_sb = avp.tile([128, 128], BF16, tag="av_sb")
            avps = ps_av.tile([128, 2, 65], F32, tag="av")
            for h in range(HPC):
                for i, kb in enumerate(kbs):
                    nc.tensor.matmul(avps[:, h, :], expt[:, h, ts(i, 128)],
                                     v_sb[:, h, b * NQB + kb, :],
                                     start=(i == 0), stop=(i == len(kbs) - 1))
                inv = avp.tile([128, 1], F32, tag="inv")
                nc.vector.reciprocal(inv[:], avps[:, h, 64:65])
                nc.vector.tensor_scalar_mul(av_sb[:, ds(h * 64, 64)],
                                            avps[:, h, 0:64], inv[:])
            avtps = ps_avt.tile([128, 128], F32, tag="avt")
            nc.tensor.matmul(avtps[:], av_sb[:], ident_sb[:], start=True, stop=True)
            nc.vector.tensor_copy(avt_sb[:, tt, :], avtps[:])
            # output projection for this token tile
            ot = outst.tile([128, D], BF16, tag="ot")
            for g in range(2):
                pt = ps_proj.tile([128, 512], F32, tag="proj")
                nc.tensor.matmul(pt[:], avt_sb[:, tt, :], woT[:, ts(g, 512)],
                                 start=True, stop=True)
                eng = nc.scalar.copy if g == 0 else nc.vector.tensor_copy
                eng(ot[:, ts(g, 512)], pt[:])
            nc.scalar.dma_start(out=out_part[ts(tt, 128), :], in_=ot[:])

        # ---- main pipeline ----
        RUNWAY = 8
        xs_tiles = {}
        for tt in range(RUNWAY):
            xs_tiles[tt] = emit_xdma(tt)
        for tt in range(RUNWAY):
            emit_xtrans(xs_tiles[tt], tt, 0)
            emit_xtrans(xs_tiles[tt], tt, 1)
            del xs_tiles[tt]
        prev = None
        for n in range(8):
            emit_qkv(n)
            for tt in range(4 * n, 4 * n + 4):
                ft = tt + RUNWAY
                if ft < NTT:
                    xs_tiles[ft] = emit_xdma(ft)
                emit_vtile(tt)
                if ft < NTT:
                    emit_xtrans(xs_tiles[ft], ft, 0)
                state = emit_scores(tt)
                if ft < NTT:
                    emit_xtrans(xs_tiles[ft], ft, 1)
                    del xs_tiles[ft]
                if prev is not None:
                    emit_avpart(prev)
                prev = state
        emit_avpart(prev)

    nc.compile()
    return nc


_NC_CACHE = {}


def _get_nc(with_bias):
    if with_bias not in _NC_CACHE:
        _NC_CACHE[with_bias] = build_nc(with_bias)
    return _NC_CACHE[with_bias]


def _make_in_maps(x, Wq, bq, Wk, bk, Wv, bv, Wo, bo):
    bf = ml_dtypes.bfloat16
    x2d = np.ascontiguousarray(np.asarray(x, np.float32).reshape(NTOK, D))
    t = np.arange(NTOK)
    tl = (t % 128).astype(np.float32)              # local position
    tb = ((t % T) - (t % 128)).astype(np.float32)  # 128*block within batch
    one = np.ones(NTOK, np.float32)
    qrows = np.stack([one, -tl, one, -tb]).astype(bf)
    krows = np.stack([tl, one, tb, one]).astype(bf)
    ident = np.eye(128, dtype=np.float32).astype(bf)
    kq = np.arange(128)
    causal = np.where(kq[:, None] <= kq[None, :], 0.0, NEG).astype(np.float32)
    scale = 1.0 / np.sqrt(np.float32(HD))

    in_maps = []
    for c in range(NCORES):
        r = slice(c * 128, (c + 1) * 128)
        in_maps.append({
            "x": x2d,
            "wqt": np.ascontiguousarray(
                (np.asarray(Wq, np.float32)[r, :] * scale).T).astype(bf)
                .reshape(8, 128, 128),
            "wkt": np.ascontiguousarray(
                np.asarray(Wk, np.float32)[r, :].T).astype(bf).reshape(8, 128, 128),
            "wvt": np.ascontiguousarray(
                np.asarray(Wv, np.float32)[r, :].T).astype(bf).reshape(8, 128, 128),
            "wot": np.ascontiguousarray(
                np.asarray(Wo, np.float32)[:, r].T).astype(bf),
            "bqr": (np.asarray(bq, np.float32)[r] * scale).reshape(1, 128).astype(bf),
            "bkr": np.asarray(bk, np.float32)[r].reshape(1, 128).astype(bf),
            "bvr": np.asarray(bv, np.float32)[r].reshape(1, 128).astype(bf),
            "ident": ident,
            "causal": causal,
            "qrows": qrows,
            "krows": krows,
        })
    return in_maps


def _assemble(results, bo):
    out = np.zeros((NTOK, D), np.float32)
    for c in range(NCORES):
        out += results[c]["out_part"].astype(np.float32)
    out += np.asarray(bo, np.float32)[None, :]
    out = out.reshape(B, T, D)

    attn = np.zeros((B, H, T, T), np.float32)
    for c in range(NCORES):
        et = results[c]["attn_t"]      # [B, HPC, T(k), T(q)] bf16, unnormalized
        for b in range(B):
            for hh in range(HPC):
                probs_t = et[b, hh].astype(np.float32)
                probs_t /= probs_t.sum(axis=0)[None, :]
                attn[b, c * HPC + hh] = probs_t.T
    return out, attn


def kernel(x, Wq, bq, Wk, bk, Wv, bv, Wo, bo, _trace=False):
    with_bias = bool(np.any(np.asarray(bq)) or np.any(np.asarray(bk))
                     or np.any(np.asarray(bv)))
    nc = _get_nc(with_bias)
    in_maps = _make_in_maps(x, Wq, bq, Wk, bk, Wv, bv, Wo, bo)
    res = run_bass_kernel_spmd(nc, in_maps, core_ids=list(range(NCORES)),
                               trace=_trace)
    out, attn = _assemble(res.results, bo)
    if _trace:
        kernel.last_exec_time_ns = res.exec_time_ns
        kernel.last_result = res
    return out, attn
